# revision 25
# baseline (speedup 1.0000x reference)
"""Multi-head causal self-attention on 8 Trainium2 NeuronCores.

Sharding: tensor-parallel over heads -- 16 heads / 8 cores = 2 heads per
core.  Every core receives the full activations x (replicated, bf16) plus
the W_Q/W_K/W_V/W_O slices for its 2 heads, computes attention + output
projection for those heads, and writes a partial (B,S,D) bf16 output.
The "all-reduce" over heads is done on the host by summing the 8
partials (b_O and the exact b_V fold are also added on the host).

v2 schedule (per core; heads h0,h1 packed on partition halves):
  - scores^T (Sk,Sq) per 128x512 block: the two heads' K=64 matmuls are
    issued back-to-back with explicit tile_position (0,0)/(64,0) so they
    CO-EXECUTE on disjoint PE row groups (measured 129 ns/MM vs 259
    serial).  Fully-masked causal blocks skipped; diagonal blocks
    column-trimmed + 0/1 mask multiply after exp (DVE).
  - exp on ScalarE only (no max subtraction: |scores/8| <= ~3, softmax is
    shift invariant).  ScalarE is the pace-setter of the attention inner
    loop (~1 elem/lane/cycle), so ALL other engines are kept off it
    except half the output-projection PSUM drains.
  - z^T (65,Sq) = V_aug.T @ expS accumulated over Sk (ones column gives
    the softmax denominators in row 64); interleaved one round behind
    scores, paced so PE never waits on exp.
  - normalize: denominator row -> DMA-reshape across 128 partitions ->
    DVE reciprocal -> DMA back to rows 0 (h0) and 64 (h1) -> the two K=1
    fp32r broadcast matmuls co-execute on disjoint row groups -> DVE
    multiply; h1's normalized z is moved to partitions 64-127 with an
    SBUF->SBUF DMA so the output projection contracts K=128 in one
    matmul per tile.
  - the output projection of round r is deferred to round r+2 and paced
    through that round's k-tile loop, so the normalize chain has a full
    round of latency slack and its PSUM drains overlap scores/z.
  - projections of batch b+1 (Q/K/V matmuls, V PE-transposes) are split
    into ~24 units and paced into batch b's attention k-tile loop: the
    attention phase is ScalarE-paced, so this fills PE idle slots; the
    batch-0 projections run as a prologue block.

All matmuls bf16 (fp32 PSUM).  fp8 was evaluated and rejected: e4m3
quantization (~3.6% RMS) on Q/K or on the attention weights pushes the
output error over the 2e-2 budget.  Output is written bf16 (partial sums
are exact-summed in fp32 on the host).
"""

import sys

import numpy as np

sys.path.insert(0, "/opt/trn_rl_repo")

# Problem dims (hardcoded per contract -- kernel.py must be self-contained).
B, S, D, H, DH = 4, 2048, 1024, 16, 64
N_CORES = 8
HPC = H // N_CORES  # heads per core = 2
SCALE = 1.0 / float(np.sqrt(DH))

NQ = 512  # q-chunk width (PSUM bank)
KT = 128  # k-tile height (partitions)


def build_program(b_dim=B, s_dim=S, d_dim=D, num_devices=N_CORES):
    """Build the per-core Bass program (same program on every core)."""
    from concourse import bacc, mybir, tile
    from concourse.masks import make_identity

    f32 = mybir.dt.float32
    f32r = mybir.dt.float32r
    bf16 = mybir.dt.bfloat16
    act = mybir.ActivationFunctionType

    KC = d_dim // 128  # contraction chunks for projections
    SQC = s_dim // NQ  # q chunks per batch
    NKT = s_dim // KT  # k tiles per batch
    RPQ = NQ // KT  # k tiles per q chunk on the diagonal (4)
    SLOTS = sum(RPQ * qc + RPQ for qc in range(SQC))  # 40 kt slots per batch

    nc = bacc.Bacc(
        "TRN2",
        target_bir_lowering=False,
        debug=False,
        enable_asserts=False,
        num_devices=num_devices,
    )

    xT = nc.dram_tensor("xT", [b_dim, d_dim, s_dim], bf16, kind="ExternalInput").ap()
    wq_d = nc.dram_tensor("wq", [128, KC, 128], bf16, kind="ExternalInput").ap()
    wk_d = nc.dram_tensor("wk", [128, KC, 128], bf16, kind="ExternalInput").ap()
    wv_d = nc.dram_tensor("wv", [128, KC, 128], bf16, kind="ExternalInput").ap()
    wo_d = nc.dram_tensor("wo", [128, d_dim], bf16, kind="ExternalInput").ap()
    bq_d = nc.dram_tensor("bq", [128, 1], f32, kind="ExternalInput").ap()
    bk_d = nc.dram_tensor("bk", [128, 1], f32, kind="ExternalInput").ap()
    masks_d = nc.dram_tensor("masks", [128, RPQ, NQ], bf16, kind="ExternalInput").ap()
    sel_d = nc.dram_tensor("sel", [2, 128], bf16, kind="ExternalInput").ap()
    out_d = nc.dram_tensor("out", [b_dim, s_dim, d_dim], bf16, kind="ExternalOutput").ap()

    with tile.TileContext(nc) as tc:
        with (
            tc.tile_pool(name="singles", bufs=1) as singles,
            tc.tile_pool(name="xpool", bufs=4 * KC) as xpool,
            tc.tile_pool(name="qkpool", bufs=2) as qkpool,
            tc.tile_pool(name="vpool", bufs=2 * NKT + 2) as vpool,
            tc.tile_pool(name="vtpool", bufs=2) as vtpool,
            tc.tile_pool(name="epool", bufs=24) as epool,
            tc.tile_pool(name="znpool", bufs=3) as znpool,
            tc.tile_pool(name="obpool", bufs=3) as obpool,
            tc.tile_pool(name="ps_s", bufs=2, space="PSUM") as ps_s,
            tc.tile_pool(name="ps_z", bufs=2, space="PSUM") as ps_z,
            tc.tile_pool(name="ps_m", bufs=2, space="PSUM") as ps_m,
        ):
            # ---- constants / weights (loaded once) ----
            wq_sb = singles.tile([128, KC, 128], bf16)
            wk_sb = singles.tile([128, KC, 128], bf16)
            wv_sb = singles.tile([128, KC, 128], bf16)
            wo_sb = singles.tile([128, d_dim], bf16)
            bq_sb = singles.tile([128, 1], f32)
            bk_sb = singles.tile([128, 1], f32)
            masks_sb = singles.tile([128, RPQ, NQ], bf16)
            ident = singles.tile([128, 128], f32)

            nc.sync.dma_start(out=wq_sb, in_=wq_d)
            nc.sync.dma_start(out=wk_sb, in_=wk_d)
            nc.sync.dma_start(out=wv_sb, in_=wv_d)
            nc.sync.dma_start(out=wo_sb, in_=wo_d)
            nc.sync.dma_start(out=bq_sb, in_=bq_d)
            nc.sync.dma_start(out=bk_sb, in_=bk_d)
            nc.sync.dma_start(out=masks_sb, in_=masks_d)
            make_identity(nc, ident)
            ident_bf = singles.tile([128, 128], bf16)
            nc.vector.tensor_copy(ident_bf, ident)
            ones_f32 = singles.tile([128, DH], f32)
            nc.vector.memset(ones_f32, 1.0)
            # head-selector for the single K=2 denominator-broadcast matmul:
            # out[0:64] gets rhs row 0 (h0 1/den), out[64:128] row 1 (h1 1/den)
            sel = singles.tile([2, 128], bf16)
            nc.sync.dma_start(out=sel, in_=sel_d)

            def stage_x(b):
                """Emit x^T load DMAs for batch b; returns [kc][q4] rhs views.

                q8-major emission order: all contraction chunks of the first
                s-half land first, so the first projection chunks can start
                at ~50% of the x transfer.
                """
                xw = min(s_dim, 2 * NQ)
                xk = [[None] * (s_dim // NQ) for _ in range(KC)]
                for q8 in range(s_dim // xw):
                    for k in range(KC):
                        xt = xpool.tile([128, xw], bf16, name=f"x_{b}_{k}_{q8}", tag="x")
                        nc.sync.dma_start(
                            out=xt,
                            in_=xT[b, k * 128 : (k + 1) * 128, q8 * xw : (q8 + 1) * xw],
                        )
                        for j in range(xw // NQ):
                            xk[k][q8 * (xw // NQ) + j] = xt[:, j * NQ : (j + 1) * NQ]
                return xk

            def make_proj_units(b, xk):
                """Q/K/V projection work for batch b as a list of closures."""
                units = []
                QTt = qkpool.tile([128, s_dim], bf16, name=f"QT_{b}", tag="QT")
                KTt = qkpool.tile([128, s_dim], bf16, name=f"KT_{b}", tag="KT")
                v_tiles = [None] * NKT
                # q4-major: everything the earliest attention rounds need
                # (and the earliest-arriving x chunks feed) is emitted first
                for q4 in range(SQC):
                    for di, (dst, wsb, bias) in enumerate(
                        ((QTt, wq_sb, bq_sb), (KTt, wk_sb, bk_sb))
                    ):
                        def u_qk(dst=dst, wsb=wsb, bias=bias, q4=q4, di=di):
                            sl = slice(q4 * NQ, (q4 + 1) * NQ)
                            pp = ps_m.tile(
                                [128, NQ], f32, name=f"pp_{b}_{di}_{q4}", tag="m"
                            )
                            for k in range(KC):
                                nc.tensor.matmul(
                                    pp,
                                    lhsT=wsb[:, k, :],
                                    rhs=xk[k][q4],
                                    start=(k == 0),
                                    stop=(k == KC - 1),
                                )
                            nc.vector.tensor_scalar_add(dst[:, sl], pp, bias)
                        units.append(u_qk)
                    box = {}
                    def u_v(q4=q4, box=box):
                        pv = ps_m.tile([128, NQ], f32, name=f"pv_{b}_{q4}", tag="m")
                        for k in range(KC):
                            nc.tensor.matmul(
                                pv,
                                lhsT=wv_sb[:, k, :],
                                rhs=xk[k][q4],
                                start=(k == 0),
                                stop=(k == KC - 1),
                            )
                        vt = vtpool.tile([128, NQ], bf16, name=f"vt_{b}_{q4}", tag="vt")
                        nc.vector.tensor_copy(vt, pv)
                        box["vt"] = vt
                    def u_t(q4=q4, box=box):
                        vt = box["vt"]
                        for j in range(NQ // 128):
                            kt = q4 * (NQ // 128) + j
                            pt = ps_m.tile([128, 128], bf16, name=f"pt_{b}_{kt}", tag="m")
                            nc.tensor.transpose(pt, vt[:, j * 128 : (j + 1) * 128], ident_bf)
                            vsb = vpool.tile(
                                [128, 2 * DH + 2], bf16, name=f"v_{b}_{kt}", tag="v"
                            )
                            nc.vector.tensor_copy(vsb[:, 0:DH], pt[:, 0:DH])
                            nc.vector.tensor_copy(
                                vsb[:, DH + 1 : 2 * DH + 1], pt[:, DH : 2 * DH]
                            )
                            nc.vector.tensor_copy(vsb[:, DH : DH + 1], ones_f32[:, 0:1])
                            nc.vector.tensor_copy(
                                vsb[:, 2 * DH + 1 : 2 * DH + 2], ones_f32[:, 0:1]
                            )
                            v_tiles[kt] = vsb
                    units.append(u_v)
                    units.append(u_t)
                return units, QTt, KTt, v_tiles

            def emit_z_pair(rec):
                zkt = rec["zkt"]
                vsb = rec["v"][zkt]
                ep, zq0 = rec["eps"][zkt]
                pz0, pz1 = rec["pz"]
                nkt_p = rec["nkt"]
                nc.tensor.matmul(
                    pz0[:, zq0:NQ],
                    lhsT=vsb[:, 0 : DH + 1],
                    rhs=ep[:, zq0:NQ],
                    start=(zkt == 0),
                    stop=(zkt == nkt_p - 1),
                )
                nc.tensor.matmul(
                    pz1[:, zq0:NQ],
                    lhsT=vsb[:, DH + 1 : 2 * DH + 2],
                    rhs=ep[:, NQ + zq0 : 2 * NQ],
                    start=(zkt == 0),
                    stop=(zkt == nkt_p - 1),
                )
                rec["zkt"] += 1

            def emit_normalize_a(rec):
                """PSUM drain + reciprocal chain of a round (no PE work).

                Builds zS (f32): h0 z on partitions 0:63, h1 z DMA-shifted to
                64:127, and denr2 (bf16): 1/den rows for both heads.
                """
                b_p, qc_p = rec["b"], rec["qc"]
                pz0, pz1 = rec["pz"]
                zS = znpool.tile([128, NQ], f32, name=f"zs_{b_p}_{qc_p}", tag="zs")
                z1 = znpool.tile([DH + 1, NQ], f32, name=f"z1_{b_p}_{qc_p}", tag="z1")
                nc.vector.tensor_copy(zS[0 : DH + 1, :], pz0)
                nc.vector.tensor_copy(z1, pz1)
                rs = znpool.tile([128, 8], f32, name=f"rs_{b_p}_{qc_p}", tag="rs")
                rr8 = znpool.tile([128, 8], bf16, name=f"rr8_{b_p}_{qc_p}", tag="rr8")
                nc.sync.dma_start(out=rs[:, 0:4], in_=zS[DH : DH + 1, :])
                nc.sync.dma_start(out=rs[:, 4:8], in_=z1[DH : DH + 1, :])
                # h1's raw z moves to partitions 64:127 (overwrites the h0
                # denominator row, already gathered into rs above)
                nc.sync.dma_start(out=zS[DH:128, :], in_=z1[0:DH, :])
                with nc.allow_low_precision("bf16 1/den feed for PE broadcast"):
                    nc.vector.reciprocal(rr8, rs)
                denr2 = znpool.tile([2, NQ], bf16, name=f"dn_{b_p}_{qc_p}", tag="dn")
                nc.sync.dma_start(out=denr2[0:1, :], in_=rr8[:, 0:4])
                nc.sync.dma_start(out=denr2[1:2, :], in_=rr8[:, 4:8])
                rec["zS"] = zS
                rec["denr2"] = denr2

            def emit_normalize_b(rec):
                """One K=2 broadcast matmul + one multiply -> normalized z."""
                b_p, qc_p = rec["b"], rec["qc"]
                pr = ps_m.tile([128, NQ], f32, name=f"pr_{b_p}_{qc_p}", tag="m")
                nc.tensor.matmul(
                    pr, lhsT=sel, rhs=rec["denr2"], start=True, stop=True
                )
                znS = znpool.tile([128, NQ], bf16, name=f"zn_{b_p}_{qc_p}", tag="zn")
                nc.vector.tensor_mul(znS, rec["zS"], pr)
                rec["znS"] = znS

            def emit_outproj_unit(rec, u):
                """One output-projection matmul (+drain, +store) of round rec."""
                b_p, qc_p = rec["b"], rec["qc"]
                mt, n2 = divmod(u, 2)
                if n2 == 0:
                    rec["ob"][mt] = obpool.tile(
                        [128, d_dim], bf16, name=f"ob_{b_p}_{qc_p}_{mt}", tag="ob"
                    )
                ob = rec["ob"][mt]
                msl = slice(mt * 128, (mt + 1) * 128)
                nsl = slice(n2 * NQ, (n2 + 1) * NQ)
                po = ps_m.tile([128, NQ], f32, name=f"po_{b_p}_{qc_p}_{mt}_{n2}", tag="m")
                nc.tensor.matmul(
                    po, lhsT=rec["znS"][:, msl], rhs=wo_sb[:, nsl], start=True, stop=True
                )
                nc.vector.tensor_copy(ob[:, nsl], po)
                if n2 == 1:
                    nc.sync.dma_start(
                        out=out_d[
                            b_p, qc_p * NQ + mt * 128 : qc_p * NQ + (mt + 1) * 128, :
                        ],
                        in_=ob,
                    )

            # ---- main pipeline over rounds (b, qc) ----
            prev = None  # round with pending z + normalize (lags 1)
            prev2 = None  # round with pending output projection (lags 2)
            proj_units = None  # next batch's projection units, paced
            proj_done = 0
            next_ctx = None

            for b in range(b_dim):
                if b == 0:
                    xk = stage_x(0)
                    units, QTt, KTt, v_tiles = make_proj_units(0, xk)
                    for u in units:
                        u()
                else:
                    QTt, KTt, v_tiles = next_ctx
                if b + 1 < b_dim:
                    xk2 = stage_x(b + 1)
                    proj_units, nQT, nKT, nv = make_proj_units(b + 1, xk2)
                    next_ctx = (nQT, nKT, nv)
                    proj_done = 0
                else:
                    proj_units = None
                slot = 0

                for qc in range(SQC):
                    nkt_q = RPQ * qc + RPQ
                    if prev is not None:
                        b_p, qc_p = prev["b"], prev["qc"]
                        prev["pz"] = (
                            ps_z.tile([DH + 1, NQ], f32, name=f"pz0_{b_p}_{qc_p}", tag="z"),
                            ps_z.tile([DH + 1, NQ], f32, name=f"pz1_{b_p}_{qc_p}", tag="z"),
                        )
                        prev["zkt"] = 0
                    opj_done = 0
                    qsl0 = qc * NQ
                    eps_cur = []

                    # k-tiles processed in PAIRS: both tiles' score matmuls are
                    # emitted as one contiguous run of four K=64 MMs (alternating
                    # row groups) so the PE two-lane pipeline stays full.
                    # The paced z/proj/outproj work is emitted BEFORE the score
                    # pair: the pair's first MM waits on exp two tiles back
                    # (PSUM ring WAR), and anything queued after it would stall
                    # behind that wait in the PE FIFO.
                    for kt0 in range(0, nkt_q, 2):
                        kt1 = kt0 + 2  # slots consumed including this pair
                        # paced: z of round-1
                        if prev is not None:
                            while (
                                prev["zkt"] < prev["nkt"]
                                and prev["zkt"] * nkt_q <= kt1 * prev["nkt"]
                            ):
                                emit_z_pair(prev)
                        # paced: next batch's projections
                        if proj_units is not None:
                            target = ((slot + 2) * len(proj_units) + SLOTS - 1) // SLOTS
                            while proj_done < min(target, len(proj_units)):
                                proj_units[proj_done]()
                                proj_done += 1
                        # round-2's normalize tail (one bcast MM) sits behind a
                        # group of sc/z/proj work so its reciprocal chain never
                        # head-of-line blocks the PE queue
                        if prev2 is not None:
                            if "znS" not in prev2:
                                emit_normalize_b(prev2)
                            # paced: output projection of round-2
                            while opj_done < 8 and opj_done * nkt_q <= kt1 * 8:
                                emit_outproj_unit(prev2, opj_done)
                                opj_done += 1
                        slot += 2
                        sps = []
                        for kt in (kt0, kt0 + 1):
                            ksl = slice(kt * KT, (kt + 1) * KT)
                            r = kt - RPQ * qc
                            q0 = 0 if r < 0 else 128 * r  # valid columns start
                            sp = ps_s.tile(
                                [128, 2 * NQ], f32, name=f"sp_{b}_{qc}_{kt}", tag="s"
                            )
                            nc.tensor.matmul(
                                sp[:, q0:NQ],
                                lhsT=KTt[0:DH, ksl],
                                rhs=QTt[0:DH, qsl0 + q0 : qsl0 + NQ],
                                start=True,
                                stop=True,
                                tile_position=(0, 0),
                            )
                            nc.tensor.matmul(
                                sp[:, NQ + q0 : 2 * NQ],
                                lhsT=KTt[DH:128, ksl],
                                rhs=QTt[DH:128, qsl0 + q0 : qsl0 + NQ],
                                start=True,
                                stop=True,
                                tile_position=(64, 0),
                            )
                            sps.append((sp, r, q0))
                        for sp, r, q0 in sps:
                            ep = epool.tile(
                                [128, 2 * NQ], bf16, name=f"ep_{b}_{qc}_{kt0}_{r}", tag="e"
                            )
                            if r < 0:
                                nc.scalar.activation(ep, sp, act.Exp, scale=SCALE)
                            else:
                                nc.scalar.activation(
                                    ep[:, q0:NQ], sp[:, q0:NQ], act.Exp, scale=SCALE
                                )
                                nc.scalar.activation(
                                    ep[:, NQ + q0 : 2 * NQ],
                                    sp[:, NQ + q0 : 2 * NQ],
                                    act.Exp,
                                    scale=SCALE,
                                )
                                nc.vector.tensor_mul(
                                    ep[:, q0:NQ], ep[:, q0:NQ], masks_sb[:, r, q0:NQ]
                                )
                                nc.vector.tensor_mul(
                                    ep[:, NQ + q0 : 2 * NQ],
                                    ep[:, NQ + q0 : 2 * NQ],
                                    masks_sb[:, r, q0:NQ],
                                )
                            eps_cur.append((ep, q0))

                    if prev is not None:
                        while prev["zkt"] < prev["nkt"]:
                            emit_z_pair(prev)
                        emit_normalize_a(prev)
                    if prev2 is not None:
                        while opj_done < 8:
                            emit_outproj_unit(prev2, opj_done)
                            opj_done += 1
                    prev2 = prev
                    prev = {
                        "b": b,
                        "qc": qc,
                        "eps": eps_cur,
                        "v": v_tiles,
                        "nkt": nkt_q,
                        "ob": [None] * (NQ // 128),
                    }

            # ---- drain: z+normalize of the last round, then its outproj.
            # outproj(prev2) covers the last normalize chain's latency ----
            b_p, qc_p = prev["b"], prev["qc"]
            prev["pz"] = (
                ps_z.tile([DH + 1, NQ], f32, name=f"pz0_{b_p}_{qc_p}", tag="z"),
                ps_z.tile([DH + 1, NQ], f32, name=f"pz1_{b_p}_{qc_p}", tag="z"),
            )
            prev["zkt"] = 0
            while prev["zkt"] < prev["nkt"]:
                emit_z_pair(prev)
            emit_normalize_a(prev)
            emit_normalize_b(prev2)
            for u in range(8):
                emit_outproj_unit(prev2, u)
            emit_normalize_b(prev)
            for u in range(8):
                emit_outproj_unit(prev, u)

    nc.compile()
    return nc


def to_bf16(a):
    import ml_dtypes

    return np.ascontiguousarray(np.asarray(a, dtype=np.float32)).astype(
        ml_dtypes.bfloat16
    )


def make_core_inputs(x, W_Q, b_Q, W_K, b_K, W_V, b_V, W_O, b_O):
    """Host-side prep: transpose x, slice + re-layout per-core weights."""
    b_dim, s_dim, d_dim = x.shape
    KC = d_dim // 128
    RPQ = NQ // KT

    xT = to_bf16(np.transpose(x, (0, 2, 1)))  # (B, D, S)

    # causal 0/1 masks for diagonal blocks, r = kt - 4*qc in 0..3
    k_idx = np.arange(KT)[:, None]
    q_idx = np.arange(NQ)[None, :]
    masks = to_bf16(
        np.stack([(q_idx >= k_idx + KT * r).astype(np.float32) for r in range(RPQ)], axis=1)
    )  # (128, RPQ, NQ)

    sel_host = np.zeros((2, 128), dtype=np.float32)
    sel_host[0, 0:DH] = 1.0
    sel_host[1, DH:128] = 1.0
    sel_host = to_bf16(sel_host)

    in_maps = []
    for c in range(N_CORES):
        h0, h1 = HPC * c, HPC * c + 1

        def stack2(w):  # (2 heads of (D, DH)) -> (128, KC, 128) chunked layout
            w2 = np.concatenate([w[h0], w[h1]], axis=1)  # (D, 128)
            return to_bf16(w2.reshape(KC, 128, 2 * DH).transpose(1, 0, 2))

        in_maps.append(
            {
                "xT": xT,
                "wq": stack2(W_Q),
                "wk": stack2(W_K),
                "wv": stack2(W_V),
                "wo": to_bf16(np.concatenate([W_O[h0], W_O[h1]], axis=0)),
                "bq": np.concatenate([b_Q[h0], b_Q[h1]]).reshape(128, 1).copy(),
                "bk": np.concatenate([b_K[h0], b_K[h1]]).reshape(128, 1).copy(),
                "masks": masks,
                "sel": sel_host,
            }
        )
    return in_maps


_PROGRAM_CACHE = {}


def run_cores(in_maps, trace=False, b_dim=B, s_dim=S, d_dim=D):
    from concourse import bass_utils

    key = (b_dim, s_dim, d_dim)
    if key not in _PROGRAM_CACHE:
        _PROGRAM_CACHE[key] = build_program(b_dim, s_dim, d_dim)
    nc = _PROGRAM_CACHE[key]
    res = bass_utils.run_bass_kernel_spmd(
        nc, in_maps, core_ids=list(range(len(in_maps))), trace=trace
    )
    return res


def kernel(x, W_Q, b_Q, W_K, b_K, W_V, b_V, W_O, b_O, _trace=False, _results=None):
    x = np.asarray(x, dtype=np.float32)
    in_maps = make_core_inputs(x, W_Q, b_Q, W_K, b_K, W_V, b_V, W_O, b_O)
    res = run_cores(in_maps, trace=_trace)
    if _results is not None:
        _results.append(res)
    out = np.zeros((B, S, D), dtype=np.float32)
    for r in res.results:
        out += np.asarray(r["out"], dtype=np.float32)
    # bias folds done on host: b_O directly; b_V's exact effect is
    # (sum_k A)=1 per head -> + sum_h b_V[h] @ W_O[h].
    out += np.asarray(b_O, dtype=np.float32)
    out += np.einsum("he,hed->d", np.asarray(b_V, np.float32), np.asarray(W_O, np.float32))
    return out


# revision 29
# speedup vs baseline: 1.0528x; 1.0528x over previous
"""Multi-head causal self-attention on 8 Trainium2 NeuronCores.

Sharding: tensor-parallel over heads -- 16 heads / 8 cores = 2 heads per
core.  Every core receives the full activations x (replicated, bf16) plus
the W_Q/W_K/W_V/W_O slices for its 2 heads, computes attention + output
projection for those heads, and writes a partial (B,S,D) bf16 output.
The "all-reduce" over heads is done on the host by summing the 8
partials (b_O and the exact b_V fold are also added on the host).

v2 schedule (per core; heads h0,h1 packed on partition halves):
  - scores^T (Sk,Sq) per 128x512 block: the two heads' K=64 matmuls are
    issued back-to-back with explicit tile_position (0,0)/(64,0) so they
    CO-EXECUTE on disjoint PE row groups (measured 129 ns/MM vs 259
    serial).  Fully-masked causal blocks skipped; diagonal blocks
    column-trimmed + 0/1 mask multiply after exp (DVE).
  - exp on ScalarE only (no max subtraction: |scores/8| <= ~3, softmax is
    shift invariant).  ScalarE is the pace-setter of the attention inner
    loop (~1 elem/lane/cycle), so ALL other engines are kept off it
    except half the output-projection PSUM drains.
  - z^T (65,Sq) = V_aug.T @ expS accumulated over Sk (ones column gives
    the softmax denominators in row 64); interleaved one round behind
    scores, paced so PE never waits on exp.
  - normalize: denominator row -> DMA-reshape across 128 partitions ->
    DVE reciprocal -> DMA back to rows 0 (h0) and 64 (h1) -> the two K=1
    fp32r broadcast matmuls co-execute on disjoint row groups -> DVE
    multiply; h1's normalized z is moved to partitions 64-127 with an
    SBUF->SBUF DMA so the output projection contracts K=128 in one
    matmul per tile.
  - the output projection of round r is deferred to round r+2 and paced
    through that round's k-tile loop, so the normalize chain has a full
    round of latency slack and its PSUM drains overlap scores/z.
  - projections of batch b+1 (Q/K/V matmuls, V PE-transposes) are split
    into ~24 units and paced into batch b's attention k-tile loop: the
    attention phase is ScalarE-paced, so this fills PE idle slots; the
    batch-0 projections run as a prologue block.

All matmuls bf16 (fp32 PSUM).  fp8 was evaluated and rejected: e4m3
quantization (~3.6% RMS) on Q/K or on the attention weights pushes the
output error over the 2e-2 budget.  Output is written bf16 (partial sums
are exact-summed in fp32 on the host).
"""

import sys

import numpy as np

sys.path.insert(0, "/opt/trn_rl_repo")

# Problem dims (hardcoded per contract -- kernel.py must be self-contained).
B, S, D, H, DH = 4, 2048, 1024, 16, 64
N_CORES = 8
HPC = H // N_CORES  # heads per core = 2
SCALE = 1.0 / float(np.sqrt(DH))

NQ = 512  # q-chunk width (PSUM bank)
KT = 128  # k-tile height (partitions)


def build_program(b_dim=B, s_dim=S, d_dim=D, num_devices=N_CORES):
    """Build the per-core Bass program (same program on every core)."""
    from concourse import bacc, mybir, tile
    from concourse.masks import make_identity

    f32 = mybir.dt.float32
    f32r = mybir.dt.float32r
    bf16 = mybir.dt.bfloat16
    act = mybir.ActivationFunctionType

    KC = d_dim // 128  # contraction chunks for projections
    SQC = s_dim // NQ  # q chunks per batch
    NKT = s_dim // KT  # k tiles per batch
    RPQ = NQ // KT  # k tiles per q chunk on the diagonal (4)
    SLOTS = sum(RPQ * qc + RPQ for qc in range(SQC))  # 40 kt slots per batch

    nc = bacc.Bacc(
        "TRN2",
        target_bir_lowering=False,
        debug=False,
        enable_asserts=False,
        num_devices=num_devices,
    )

    xT = nc.dram_tensor("xT", [b_dim, d_dim, s_dim], bf16, kind="ExternalInput").ap()
    wq_d = nc.dram_tensor("wq", [128, KC, 128], bf16, kind="ExternalInput").ap()
    wk_d = nc.dram_tensor("wk", [128, KC, 128], bf16, kind="ExternalInput").ap()
    wv_d = nc.dram_tensor("wv", [128, KC, 128], bf16, kind="ExternalInput").ap()
    wo_d = nc.dram_tensor("wo", [128, d_dim], bf16, kind="ExternalInput").ap()
    bq_d = nc.dram_tensor("bq", [128, 1], f32, kind="ExternalInput").ap()
    bk_d = nc.dram_tensor("bk", [128, 1], f32, kind="ExternalInput").ap()
    masks_d = nc.dram_tensor("masks", [128, RPQ, NQ], bf16, kind="ExternalInput").ap()
    sel_d = nc.dram_tensor("sel", [2, 128], bf16, kind="ExternalInput").ap()
    out_d = nc.dram_tensor("out", [b_dim, s_dim, d_dim], bf16, kind="ExternalOutput").ap()

    with tile.TileContext(nc) as tc:
        with (
            tc.tile_pool(name="singles", bufs=1) as singles,
            tc.tile_pool(name="xpool", bufs=4 * KC) as xpool,
            tc.tile_pool(name="qkpool", bufs=2) as qkpool,
            tc.tile_pool(name="vpool", bufs=2 * NKT + 2) as vpool,
            tc.tile_pool(name="vtpool", bufs=2) as vtpool,
            tc.tile_pool(name="epool", bufs=24) as epool,
            tc.tile_pool(name="znpool", bufs=3) as znpool,
            tc.tile_pool(name="obpool", bufs=3) as obpool,
            tc.tile_pool(name="ps_s", bufs=2, space="PSUM") as ps_s,
            tc.tile_pool(name="ps_z", bufs=2, space="PSUM") as ps_z,
            tc.tile_pool(name="ps_m", bufs=2, space="PSUM") as ps_m,
        ):
            # ---- constants / weights (loaded once) ----
            wq_sb = singles.tile([128, KC, 128], bf16)
            wk_sb = singles.tile([128, KC, 128], bf16)
            wv_sb = singles.tile([128, KC, 128], bf16)
            wo_sb = singles.tile([128, d_dim], bf16)
            bq_sb = singles.tile([128, 1], f32)
            bk_sb = singles.tile([128, 1], f32)
            masks_sb = singles.tile([128, RPQ, NQ], bf16)
            ident = singles.tile([128, 128], f32)

            nc.sync.dma_start(out=wq_sb, in_=wq_d)
            nc.sync.dma_start(out=wk_sb, in_=wk_d)
            nc.sync.dma_start(out=wv_sb, in_=wv_d)
            nc.sync.dma_start(out=wo_sb, in_=wo_d)
            nc.sync.dma_start(out=bq_sb, in_=bq_d)
            nc.sync.dma_start(out=bk_sb, in_=bk_d)
            nc.sync.dma_start(out=masks_sb, in_=masks_d)
            make_identity(nc, ident)
            ident_bf = singles.tile([128, 128], bf16)
            nc.vector.tensor_copy(ident_bf, ident)
            ones_f32 = singles.tile([128, DH], f32)
            nc.vector.memset(ones_f32, 1.0)
            # head-selector for the single K=2 denominator-broadcast matmul:
            # out[0:64] gets rhs row 0 (h0 1/den), out[64:128] row 1 (h1 1/den)
            sel = singles.tile([2, 128], bf16)
            nc.sync.dma_start(out=sel, in_=sel_d)

            def stage_x(b):
                """Emit x^T load DMAs for batch b; returns [kc][q4] rhs views.

                q8-major emission order: all contraction chunks of the first
                s-half land first, so the first projection chunks can start
                at ~50% of the x transfer.
                """
                xw = min(s_dim, 2 * NQ)
                xk = [[None] * (s_dim // NQ) for _ in range(KC)]
                for q8 in range(s_dim // xw):
                    for k in range(KC):
                        xt = xpool.tile([128, xw], bf16, name=f"x_{b}_{k}_{q8}", tag="x")
                        nc.sync.dma_start(
                            out=xt,
                            in_=xT[b, k * 128 : (k + 1) * 128, q8 * xw : (q8 + 1) * xw],
                        )
                        for j in range(xw // NQ):
                            xk[k][q8 * (xw // NQ) + j] = xt[:, j * NQ : (j + 1) * NQ]
                return xk

            def make_proj_units(b, xk):
                """Q/K/V projection work for batch b as a list of closures."""
                units = []
                QTt = qkpool.tile([128, s_dim], bf16, name=f"QT_{b}", tag="QT")
                KTt = qkpool.tile([128, s_dim], bf16, name=f"KT_{b}", tag="KT")
                v_tiles = [None] * NKT
                # q4-major: everything the earliest attention rounds need
                # (and the earliest-arriving x chunks feed) is emitted first
                for q4 in range(SQC):
                    for di, (dst, wsb, bias) in enumerate(
                        ((QTt, wq_sb, bq_sb), (KTt, wk_sb, bk_sb))
                    ):
                        def u_qk(dst=dst, wsb=wsb, bias=bias, q4=q4, di=di):
                            sl = slice(q4 * NQ, (q4 + 1) * NQ)
                            pp = ps_m.tile(
                                [128, NQ], f32, name=f"pp_{b}_{di}_{q4}", tag="m"
                            )
                            for k in range(KC):
                                nc.tensor.matmul(
                                    pp,
                                    lhsT=wsb[:, k, :],
                                    rhs=xk[k][q4],
                                    start=(k == 0),
                                    stop=(k == KC - 1),
                                )
                            nc.vector.tensor_scalar_add(dst[:, sl], pp, bias)
                        units.append(u_qk)
                    box = {}
                    def u_v(q4=q4, box=box):
                        pv = ps_m.tile([128, NQ], f32, name=f"pv_{b}_{q4}", tag="m")
                        for k in range(KC):
                            nc.tensor.matmul(
                                pv,
                                lhsT=wv_sb[:, k, :],
                                rhs=xk[k][q4],
                                start=(k == 0),
                                stop=(k == KC - 1),
                            )
                        vt = vtpool.tile([128, NQ], bf16, name=f"vt_{b}_{q4}", tag="vt")
                        nc.vector.tensor_copy(vt, pv)
                        box["vt"] = vt
                    def u_t(q4=q4, box=box):
                        vt = box["vt"]
                        for j in range(NQ // 128):
                            kt = q4 * (NQ // 128) + j
                            pt = ps_m.tile([128, 128], bf16, name=f"pt_{b}_{kt}", tag="m")
                            nc.tensor.transpose(pt, vt[:, j * 128 : (j + 1) * 128], ident_bf)
                            vsb = vpool.tile(
                                [128, 2 * DH + 2], bf16, name=f"v_{b}_{kt}", tag="v"
                            )
                            nc.vector.tensor_copy(vsb[:, 0:DH], pt[:, 0:DH])
                            nc.vector.tensor_copy(
                                vsb[:, DH + 1 : 2 * DH + 1], pt[:, DH : 2 * DH]
                            )
                            nc.vector.tensor_copy(vsb[:, DH : DH + 1], ones_f32[:, 0:1])
                            nc.vector.tensor_copy(
                                vsb[:, 2 * DH + 1 : 2 * DH + 2], ones_f32[:, 0:1]
                            )
                            v_tiles[kt] = vsb
                    units.append(u_v)
                    units.append(u_t)
                return units, QTt, KTt, v_tiles

            def emit_z_pair(rec):
                zkt = rec["zkt"]
                vsb = rec["v"][zkt]
                ep, zq0 = rec["eps"][zkt]
                pz0, pz1 = rec["pz"]
                nkt_p = rec["nkt"]
                nc.tensor.matmul(
                    pz0[:, zq0:NQ],
                    lhsT=vsb[:, 0 : DH + 1],
                    rhs=ep[:, zq0:NQ],
                    start=(zkt == 0),
                    stop=(zkt == nkt_p - 1),
                )
                nc.tensor.matmul(
                    pz1[:, zq0:NQ],
                    lhsT=vsb[:, DH + 1 : 2 * DH + 2],
                    rhs=ep[:, NQ + zq0 : 2 * NQ],
                    start=(zkt == 0),
                    stop=(zkt == nkt_p - 1),
                )
                rec["zkt"] += 1

            def emit_normalize_a(rec):
                """PSUM drain + reciprocal chain of a round (no PE work).

                Builds zS (f32): h0 z on partitions 0:63, h1 z DMA-shifted to
                64:127, and denr2 (bf16): 1/den rows for both heads.
                """
                b_p, qc_p = rec["b"], rec["qc"]
                pz0, pz1 = rec["pz"]
                zS = znpool.tile([128, NQ], f32, name=f"zs_{b_p}_{qc_p}", tag="zs")
                z1 = znpool.tile([DH + 1, NQ], f32, name=f"z1_{b_p}_{qc_p}", tag="z1")
                nc.vector.tensor_copy(zS[0 : DH + 1, :], pz0)
                nc.vector.tensor_copy(z1, pz1)
                rs = znpool.tile([128, 8], f32, name=f"rs_{b_p}_{qc_p}", tag="rs")
                rr8 = znpool.tile([128, 8], bf16, name=f"rr8_{b_p}_{qc_p}", tag="rr8")
                nc.sync.dma_start(out=rs[:, 0:4], in_=zS[DH : DH + 1, :])
                nc.sync.dma_start(out=rs[:, 4:8], in_=z1[DH : DH + 1, :])
                # h1's raw z moves to partitions 64:127 (overwrites the h0
                # denominator row, already gathered into rs above)
                nc.sync.dma_start(out=zS[DH:128, :], in_=z1[0:DH, :])
                with nc.allow_low_precision("bf16 1/den feed for PE broadcast"):
                    nc.vector.reciprocal(rr8, rs)
                denr2 = znpool.tile([2, NQ], bf16, name=f"dn_{b_p}_{qc_p}", tag="dn")
                nc.sync.dma_start(out=denr2[0:1, :], in_=rr8[:, 0:4])
                nc.sync.dma_start(out=denr2[1:2, :], in_=rr8[:, 4:8])
                rec["zS"] = zS
                rec["denr2"] = denr2

            def emit_normalize_b(rec):
                """One K=2 broadcast matmul + one multiply -> normalized z."""
                b_p, qc_p = rec["b"], rec["qc"]
                pr = ps_m.tile([128, NQ], f32, name=f"pr_{b_p}_{qc_p}", tag="m")
                nc.tensor.matmul(
                    pr, lhsT=sel, rhs=rec["denr2"], start=True, stop=True
                )
                znS = znpool.tile([128, NQ], bf16, name=f"zn_{b_p}_{qc_p}", tag="zn")
                nc.vector.tensor_mul(znS, rec["zS"], pr)
                rec["znS"] = znS

            def emit_outproj_unit(rec, u):
                """One output-projection matmul (+drain, +store) of round rec."""
                b_p, qc_p = rec["b"], rec["qc"]
                mt, n2 = divmod(u, 2)
                if n2 == 0:
                    rec["ob"][mt] = obpool.tile(
                        [128, d_dim], bf16, name=f"ob_{b_p}_{qc_p}_{mt}", tag="ob"
                    )
                ob = rec["ob"][mt]
                msl = slice(mt * 128, (mt + 1) * 128)
                nsl = slice(n2 * NQ, (n2 + 1) * NQ)
                po = ps_m.tile([128, NQ], f32, name=f"po_{b_p}_{qc_p}_{mt}_{n2}", tag="m")
                nc.tensor.matmul(
                    po, lhsT=rec["znS"][:, msl], rhs=wo_sb[:, nsl], start=True, stop=True
                )
                nc.vector.tensor_copy(ob[:, nsl], po)
                if n2 == 1:
                    nc.sync.dma_start(
                        out=out_d[
                            b_p, qc_p * NQ + mt * 128 : qc_p * NQ + (mt + 1) * 128, :
                        ],
                        in_=ob,
                    )

            # ---- main pipeline over rounds (b, qc) ----
            prev = None  # round with pending z + normalize (lags 1)
            prev2 = None  # round with pending output projection (lags 2)
            next_ctx = None
            # batch b+1's projections pace over batch b's attention AND spill
            # into b+1's own early rounds (deadline: chunk q4 must land before
            # round (b+1, q4) starts) -- this leaves PE work to fill the last
            # batch's exp-paced attention slots
            round_start = [sum(RPQ * q + RPQ for q in range(qc)) for qc in range(SQC)]
            proj_states = []  # list of dicts: units, done, start_gslot
            gslot = 0

            def pump_proj(gs):
                for st in proj_states:
                    units = st["units"]
                    window = SLOTS + round_start[SQC - 1] + RPQ  # SLOTS + 28
                    rel = gs - st["start"] + 2
                    target = (rel * len(units) + window - 1) // window
                    while st["done"] < min(target, len(units)):
                        units[st["done"]]()
                        st["done"] += 1
                    # deadline enforcement: unit i (chunk q4 = i//4) before
                    # round (b+1, q4) begins
                    while st["done"] < len(units):
                        dl = SLOTS + round_start[min(st["done"] // 4, SQC - 1)]
                        if rel > dl:
                            units[st["done"]]()
                            st["done"] += 1
                        else:
                            break
                proj_states[:] = [s for s in proj_states if s["done"] < len(s["units"])]

            for b in range(b_dim):
                if b == 0:
                    xk = stage_x(0)
                    units, QTt, KTt, v_tiles = make_proj_units(0, xk)
                    for u in units:
                        u()
                else:
                    QTt, KTt, v_tiles = next_ctx
                if b + 1 < b_dim:
                    xk2 = stage_x(b + 1)
                    nunits, nQT, nKT, nv = make_proj_units(b + 1, xk2)
                    next_ctx = (nQT, nKT, nv)
                    proj_states.append({"units": nunits, "done": 0, "start": gslot})

                for qc in range(SQC):
                    nkt_q = RPQ * qc + RPQ
                    if prev is not None:
                        b_p, qc_p = prev["b"], prev["qc"]
                        prev["pz"] = (
                            ps_z.tile([DH + 1, NQ], f32, name=f"pz0_{b_p}_{qc_p}", tag="z"),
                            ps_z.tile([DH + 1, NQ], f32, name=f"pz1_{b_p}_{qc_p}", tag="z"),
                        )
                        prev["zkt"] = 0
                    opj_done = 0
                    qsl0 = qc * NQ
                    eps_cur = []

                    # k-tiles processed in PAIRS: both tiles' score matmuls are
                    # emitted as one contiguous run of four K=64 MMs (alternating
                    # row groups) so the PE two-lane pipeline stays full.
                    # The paced z/proj/outproj work is emitted BEFORE the score
                    # pair: the pair's first MM waits on exp two tiles back
                    # (PSUM ring WAR), and anything queued after it would stall
                    # behind that wait in the PE FIFO.
                    for kt0 in range(0, nkt_q, 2):
                        kt1 = kt0 + 2  # slots consumed including this pair
                        # paced: z of round-1
                        if prev is not None:
                            while (
                                prev["zkt"] < prev["nkt"]
                                and prev["zkt"] * nkt_q <= kt1 * prev["nkt"]
                            ):
                                emit_z_pair(prev)
                        # paced: projections (next batch's, possibly spilled)
                        pump_proj(gslot)
                        # round-2's normalize tail (one bcast MM) sits behind a
                        # group of sc/z/proj work so its reciprocal chain never
                        # head-of-line blocks the PE queue
                        if prev2 is not None:
                            if "znS" not in prev2:
                                emit_normalize_b(prev2)
                            # paced: output projection of round-2
                            while opj_done < 8 and opj_done * nkt_q <= kt1 * 8:
                                emit_outproj_unit(prev2, opj_done)
                                opj_done += 1
                        gslot += 2
                        sps = []
                        for kt in (kt0, kt0 + 1):
                            ksl = slice(kt * KT, (kt + 1) * KT)
                            r = kt - RPQ * qc
                            q0 = 0 if r < 0 else 128 * r  # valid columns start
                            sp = ps_s.tile(
                                [128, 2 * NQ], f32, name=f"sp_{b}_{qc}_{kt}", tag="s"
                            )
                            nc.tensor.matmul(
                                sp[:, q0:NQ],
                                lhsT=KTt[0:DH, ksl],
                                rhs=QTt[0:DH, qsl0 + q0 : qsl0 + NQ],
                                start=True,
                                stop=True,
                                tile_position=(0, 0),
                            )
                            nc.tensor.matmul(
                                sp[:, NQ + q0 : 2 * NQ],
                                lhsT=KTt[DH:128, ksl],
                                rhs=QTt[DH:128, qsl0 + q0 : qsl0 + NQ],
                                start=True,
                                stop=True,
                                tile_position=(64, 0),
                            )
                            sps.append((sp, r, q0))
                        for sp, r, q0 in sps:
                            ep = epool.tile(
                                [128, 2 * NQ], bf16, name=f"ep_{b}_{qc}_{kt0}_{r}", tag="e"
                            )
                            if r < 0:
                                nc.scalar.activation(ep, sp, act.Exp, scale=SCALE)
                            else:
                                nc.scalar.activation(
                                    ep[:, q0:NQ], sp[:, q0:NQ], act.Exp, scale=SCALE
                                )
                                nc.scalar.activation(
                                    ep[:, NQ + q0 : 2 * NQ],
                                    sp[:, NQ + q0 : 2 * NQ],
                                    act.Exp,
                                    scale=SCALE,
                                )
                                nc.vector.tensor_mul(
                                    ep[:, q0:NQ], ep[:, q0:NQ], masks_sb[:, r, q0:NQ]
                                )
                                nc.vector.tensor_mul(
                                    ep[:, NQ + q0 : 2 * NQ],
                                    ep[:, NQ + q0 : 2 * NQ],
                                    masks_sb[:, r, q0:NQ],
                                )
                            eps_cur.append((ep, q0))

                    if prev is not None:
                        while prev["zkt"] < prev["nkt"]:
                            emit_z_pair(prev)
                        emit_normalize_a(prev)
                    if prev2 is not None:
                        while opj_done < 8:
                            emit_outproj_unit(prev2, opj_done)
                            opj_done += 1
                    prev2 = prev
                    prev = {
                        "b": b,
                        "qc": qc,
                        "eps": eps_cur,
                        "v": v_tiles,
                        "nkt": nkt_q,
                        "ob": [None] * (NQ // 128),
                    }

            # ---- drain: z+normalize of the last round, then its outproj.
            # outproj(prev2) covers the last normalize chain's latency ----
            b_p, qc_p = prev["b"], prev["qc"]
            prev["pz"] = (
                ps_z.tile([DH + 1, NQ], f32, name=f"pz0_{b_p}_{qc_p}", tag="z"),
                ps_z.tile([DH + 1, NQ], f32, name=f"pz1_{b_p}_{qc_p}", tag="z"),
            )
            prev["zkt"] = 0
            while prev["zkt"] < prev["nkt"]:
                emit_z_pair(prev)
            emit_normalize_a(prev)
            emit_normalize_b(prev2)
            for u in range(8):
                emit_outproj_unit(prev2, u)
            emit_normalize_b(prev)
            for u in range(8):
                emit_outproj_unit(prev, u)

    nc.compile()
    return nc


def to_bf16(a):
    import ml_dtypes

    return np.ascontiguousarray(np.asarray(a, dtype=np.float32)).astype(
        ml_dtypes.bfloat16
    )


def make_core_inputs(x, W_Q, b_Q, W_K, b_K, W_V, b_V, W_O, b_O):
    """Host-side prep: transpose x, slice + re-layout per-core weights."""
    b_dim, s_dim, d_dim = x.shape
    KC = d_dim // 128
    RPQ = NQ // KT

    xT = to_bf16(np.transpose(x, (0, 2, 1)))  # (B, D, S)

    # causal 0/1 masks for diagonal blocks, r = kt - 4*qc in 0..3
    k_idx = np.arange(KT)[:, None]
    q_idx = np.arange(NQ)[None, :]
    masks = to_bf16(
        np.stack([(q_idx >= k_idx + KT * r).astype(np.float32) for r in range(RPQ)], axis=1)
    )  # (128, RPQ, NQ)

    sel_host = np.zeros((2, 128), dtype=np.float32)
    sel_host[0, 0:DH] = 1.0
    sel_host[1, DH:128] = 1.0
    sel_host = to_bf16(sel_host)

    in_maps = []
    for c in range(N_CORES):
        h0, h1 = HPC * c, HPC * c + 1

        def stack2(w):  # (2 heads of (D, DH)) -> (128, KC, 128) chunked layout
            w2 = np.concatenate([w[h0], w[h1]], axis=1)  # (D, 128)
            return to_bf16(w2.reshape(KC, 128, 2 * DH).transpose(1, 0, 2))

        in_maps.append(
            {
                "xT": xT,
                "wq": stack2(W_Q),
                "wk": stack2(W_K),
                "wv": stack2(W_V),
                "wo": to_bf16(np.concatenate([W_O[h0], W_O[h1]], axis=0)),
                "bq": np.concatenate([b_Q[h0], b_Q[h1]]).reshape(128, 1).copy(),
                "bk": np.concatenate([b_K[h0], b_K[h1]]).reshape(128, 1).copy(),
                "masks": masks,
                "sel": sel_host,
            }
        )
    return in_maps


_PROGRAM_CACHE = {}


def run_cores(in_maps, trace=False, b_dim=B, s_dim=S, d_dim=D):
    from concourse import bass_utils

    key = (b_dim, s_dim, d_dim)
    if key not in _PROGRAM_CACHE:
        _PROGRAM_CACHE[key] = build_program(b_dim, s_dim, d_dim)
    nc = _PROGRAM_CACHE[key]
    res = bass_utils.run_bass_kernel_spmd(
        nc, in_maps, core_ids=list(range(len(in_maps))), trace=trace
    )
    return res


def kernel(x, W_Q, b_Q, W_K, b_K, W_V, b_V, W_O, b_O, _trace=False, _results=None):
    x = np.asarray(x, dtype=np.float32)
    in_maps = make_core_inputs(x, W_Q, b_Q, W_K, b_K, W_V, b_V, W_O, b_O)
    res = run_cores(in_maps, trace=_trace)
    if _results is not None:
        _results.append(res)
    out = np.zeros((B, S, D), dtype=np.float32)
    for r in res.results:
        out += np.asarray(r["out"], dtype=np.float32)
    # bias folds done on host: b_O directly; b_V's exact effect is
    # (sum_k A)=1 per head -> + sum_h b_V[h] @ W_O[h].
    out += np.asarray(b_O, dtype=np.float32)
    out += np.einsum("he,hed->d", np.asarray(b_V, np.float32), np.asarray(W_O, np.float32))
    return out
